# revision 61
# baseline (speedup 1.0000x reference)
import os
import sys

import numpy as np

for _p in ("/opt/trn_rl_repo",):
    if os.path.isdir(_p) and _p not in sys.path:
        sys.path.insert(0, _p)

import concourse.bass as bass
import concourse.tile as tile
from concourse import bacc, mybir
from concourse.alu_op_type import AluOpType
from concourse.bass_utils import run_bass_kernel_spmd

F32 = mybir.dt.float32
F32R = mybir.dt.float32r
BF16 = mybir.dt.bfloat16
I32 = mybir.dt.int32
AF = mybir.ActivationFunctionType

B, N, C, H = 8, 4096, 256, 8
D = C // H
HID = 4 * C
EPS = 1e-5
P = 128
SS = 512           # tokens per superstep
NSS = N // SS      # 8 supersteps
NST = SS // P      # 4 token subtiles per superstep
KC = C // P        # 2 channel chunks
HC = HID // P      # 8 hidden chunks
MAGIC = 0x5F3759DF

LAST_RESULT = None  # test.py reads exec_time_ns / profile from here


def _build(nz):
    """Per-core Bass program.

    Layout conventions:
      token-major tile: [128 tok-partition, NST, C]
      channel-major (transposed): [128 ch-partition, KC, SS]
    LN folded as: x_ln_stored = (x - mu) * rsqrt(M2 + C*eps); the *16
    (= sqrt(C)) factor is folded into the consuming weights host-side.
    Gate sigmoid(g) = 0.5*tanh(0.5 g) + 0.5; the 0.5 input scale is folded
    into the gate weights host-side.
    """
    nc = bacc.Bacc("TRN2", target_bir_lowering=False, debug=False, num_devices=8)

    x_d = nc.dram_tensor("x", [N, C], F32, kind="ExternalInput").ap()
    s_d = nc.dram_tensor("s", [N, C], F32, kind="ExternalInput").ap()
    pos_d = nc.dram_tensor("pos", [N, C], F32, kind="ExternalInput").ap()
    wqi_d = nc.dram_tensor("w_qi", [C, 3 * C], BF16, kind="ExternalInput").ap()
    wqs_d = nc.dram_tensor("w_qs", [C, 3 * C], BF16, kind="ExternalInput").ap()
    wproj_d = nc.dram_tensor("w_proj", [C, C], BF16, kind="ExternalInput").ap()
    wgate_d = nc.dram_tensor("w_gate", [2 * C, C], BF16, kind="ExternalInput").ap()
    wfc1_d = nc.dram_tensor("w_fc1", [C, HID], BF16, kind="ExternalInput").ap()
    wfc2_d = nc.dram_tensor("w_fc2", [HID, C], BF16, kind="ExternalInput").ap()
    mask_d = nc.dram_tensor("mask", [C, C], BF16, kind="ExternalInput").ap()
    ident_d = nc.dram_tensor("ident", [P, P], BF16, kind="ExternalInput").ap()
    magic_d = nc.dram_tensor("magic", [16], I32, kind="ExternalInput").ap()
    bq_d = nc.dram_tensor("b_q", [C], F32, kind="ExternalInput").ap() if nz["b_q"] else None
    bkv_d = nc.dram_tensor("b_kv", [2 * C], F32, kind="ExternalInput").ap() if nz["b_kv"] else None
    bproj_d = nc.dram_tensor("b_proj", [C], F32, kind="ExternalInput").ap() if nz["b_proj"] else None
    bgate_d = nc.dram_tensor("b_gate", [C], F32, kind="ExternalInput").ap() if nz["b_gate"] else None
    bfc1_d = nc.dram_tensor("b_fc1", [HID], F32, kind="ExternalInput").ap() if nz["b_fc1"] else None
    bfc2_d = nc.dram_tensor("b_fc2", [C], F32, kind="ExternalInput").ap() if nz["b_fc2"] else None

    out_d = nc.dram_tensor("out", [N, C], F32, kind="ExternalOutput").ap()
    ns_d = nc.dram_tensor("ns", [N, C], F32, kind="ExternalOutput").ap()

    dbg = os.environ.get("KBDG", "0") == "1"
    if dbg:
        dbg_phiqt_d = nc.dram_tensor("dbg_phiqt", [P, KC * N], BF16, kind="ExternalOutput").ap()
        dbg_kvbd_d = nc.dram_tensor("dbg_kvbd", [P, KC * C], BF16, kind="ExternalOutput").ap()
        dbg_m_d = nc.dram_tensor("dbg_m", [P, KC * C], BF16, kind="ExternalOutput").ap()
        dbg_upd_d = nc.dram_tensor("dbg_upd", [P, (N // P) * C], BF16, kind="ExternalOutput").ap()
        dbg_xp_d = nc.dram_tensor("dbg_xp", [P, (N // P) * C], BF16, kind="ExternalOutput").ap()
        dbg_phik_d = nc.dram_tensor("dbg_phik", [P, NST * C], BF16, kind="ExternalOutput").ap()
        dbg_v_d = nc.dram_tensor("dbg_v", [P, NST * C], BF16, kind="ExternalOutput").ap()

    def bcast_row(vec_ap, n):
        return bass.AP(
            tensor=vec_ap.tensor, offset=vec_ap.offset, ap=[[0, P]] + vec_ap.ap
        )

    with tile.TileContext(nc) as tc:
        with (
            tc.tile_pool(name="wts", bufs=1) as wts,
            tc.tile_pool(name="pers", bufs=1) as pers,
            tc.tile_pool(name="io", bufs=2) as io,
            tc.tile_pool(name="wk", bufs=2) as wk,
            tc.tile_pool(name="sm", bufs=3) as sm,
            tc.tile_pool(name="pmm", bufs=5, space="PSUM") as pmm,
            tc.tile_pool(name="ptpa", bufs=2, space="PSUM") as ptpa,
            tc.tile_pool(name="pkv", bufs=1, space="PSUM") as pkv,
        ):
            # ---- weights / constants ----
            # Order matters: phase-1-critical small constants first so the
            # first superstep isn't blocked behind the big weight loads;
            # phase-2-only weights (wproj/wfc1/wfc2/mask) go last.
            ident = wts.tile([P, P], BF16, name="ident")
            nc.sync.dma_start(out=ident, in_=ident_d)
            magic = wts.tile([P, 16], I32, name="magic")
            nc.sync.dma_start(out=magic, in_=bcast_row(magic_d, 16))

            # prefetch superstep-0 inputs ahead of the big weight DMAs
            pre_x = io.tile([P, NST, C], F32, name="prex", tag="xt")
            nc.sync.dma_start(out=pre_x, in_=x_d[0:SS, :].rearrange("(s p) c -> p s c", p=P))
            pre_s = io.tile([P, NST, C], F32, name="pres", tag="st", bufs=3)
            nc.sync.dma_start(out=pre_s, in_=s_d[0:SS, :].rearrange("(s p) c -> p s c", p=P))
            pre_p = io.tile([P, NST, C], F32, name="prep", tag="pt")
            nc.sync.dma_start(out=pre_p, in_=pos_d[0:SS, :].rearrange("(s p) c -> p s c", p=P))
            wqi = wts.tile([P, KC, 3 * C], BF16, name="wqi")
            nc.sync.dma_start(out=wqi, in_=wqi_d.rearrange("(k p) o -> p k o", p=P))
            wqs = wts.tile([P, KC, 3 * C], BF16, name="wqs")
            nc.sync.dma_start(out=wqs, in_=wqs_d.rearrange("(k p) o -> p k o", p=P))
            wgate = wts.tile([P, 4, C], BF16, name="wgate")
            nc.sync.dma_start(out=wgate, in_=wgate_d.rearrange("(k p) o -> p k o", p=P))
            wproj = wts.tile([P, KC, C], BF16, name="wproj")
            nc.sync.dma_start(out=wproj, in_=wproj_d.rearrange("(k p) o -> p k o", p=P))
            wfc1 = wts.tile([P, KC, HID], BF16, name="wfc1")
            nc.sync.dma_start(out=wfc1, in_=wfc1_d.rearrange("(k p) o -> p k o", p=P))
            wfc2 = wts.tile([P, HC, C], BF16, name="wfc2")
            nc.sync.dma_start(out=wfc2, in_=wfc2_d.rearrange("(k p) o -> p k o", p=P))
            mask = wts.tile([P, KC, C], BF16, name="mask")
            nc.sync.dma_start(out=mask, in_=mask_d.rearrange("(k p) o -> p k o", p=P))

            bq_sb = None
            if bq_d is not None:
                bq_sb = wts.tile([P, KC], F32, name="bqsb")
                nc.sync.dma_start(out=bq_sb, in_=bq_d.rearrange("(k p) -> p k", p=P))
            bkv_sb = None
            if bkv_d is not None:
                bkv_sb = wts.tile([P, 2 * C], F32, name="bkvsb")
                nc.sync.dma_start(out=bkv_sb, in_=bcast_row(bkv_d, 2 * C))
            bproj_sb = None
            if bproj_d is not None:
                bproj_sb = wts.tile([P, C], F32, name="bprojsb")
                nc.sync.dma_start(out=bproj_sb, in_=bcast_row(bproj_d, C))
            bgate_sb = None
            if bgate_d is not None:
                bgate_sb = wts.tile([P, C], F32, name="bgatesb")
                nc.sync.dma_start(out=bgate_sb, in_=bcast_row(bgate_d, C))
            bfc1_sb = None
            if bfc1_d is not None:
                bfc1_sb = wts.tile([P, HC], F32, name="bfc1sb")
                nc.sync.dma_start(out=bfc1_sb, in_=bfc1_d.rearrange("(k p) -> p k", p=P))
            bfc2_sb = None
            if bfc2_d is not None:
                bfc2_sb = wts.tile([P, C], F32, name="bfc2sb")
                nc.sync.dma_start(out=bfc2_sb, in_=bcast_row(bfc2_d, C))

            # ---- persistent state ----
            xp_full = pers.tile([P, N // P, C], BF16, name="xpfull")
            phiqT = pers.tile([P, KC, N], BF16, name="phiqt")
            upd = pers.tile([P, N // P, C], BF16, name="updfull")

            kvt = pkv.tile([P, KC, C], F32, name="kvt")
            kvps = [kvt[:, 0, :], kvt[:, 1, :]]
            # start=True clears has_written for the WHOLE bank, so the two
            # interleaved accumulation groups sharing this bank must be
            # started by a single bank-wide zeroing matmul; all real kv
            # matmuls then accumulate with start=False.
            zlhs = wts.tile([P, P], BF16, name="zlhs")
            nc.vector.memset(zlhs, 0.0)
            kvflat = kvt.rearrange("p a b -> p (a b)")
            for zi in range(4):
                nc.tensor.matmul(
                    kvflat[:, zi * P:(zi + 1) * P],
                    lhsT=zlhs,
                    rhs=zlhs,
                    start=True, stop=False,
                    skip_group_check=True,
                )

            def stats_chain(stt, mu, rr, ngroups):
                """stt [P, G, 6] bn_stats out -> mu [P, G], rr [P, G] with
                rr = rsqrt(M2_total + C*eps) (Newton w/ magic seed).
                Layout: [cnt_e, mean_e, M2_e, cnt_o, mean_o, M2_o].
                Float merge ops run on Pool; int seed + Newton on DVE."""
                m_e = stt[:, :, 1]
                m2e = stt[:, :, 2]
                m_o = stt[:, :, 4]
                m2o = stt[:, :, 5]
                dd = sm.tile([P, ngroups], F32, name="dd")
                nc.vector.tensor_sub(dd, m_e, m_o)
                nc.vector.tensor_mul(dd, dd, dd)
                m2s = sm.tile([P, ngroups], F32, name="m2s")
                nc.vector.tensor_add(m2s, m2e, m2o)
                q = sm.tile([P, ngroups], F32, name="qq")
                # q = dd*(C/4) + m2s + C*eps  (merge: M2 = M2_e+M2_o + (C/4)d^2)
                nc.vector.scalar_tensor_tensor(
                    q, dd, float(C) / 4.0, m2s, AluOpType.mult, AluOpType.add
                )
                nc.vector.tensor_scalar_add(q, q, float(C) * EPS)
                # Newton rsqrt with quake seed
                hshift = sm.tile([P, ngroups], I32, name="hshift")
                nc.vector.tensor_scalar(
                    hshift, q.bitcast(I32), 1, None, AluOpType.logical_shift_right
                )
                y0 = sm.tile([P, ngroups], F32, name="y0")
                nc.vector.tensor_sub(y0.bitcast(I32), magic[:, 0:ngroups], hshift)
                t1 = sm.tile([P, ngroups], F32, name="t1")
                nc.vector.tensor_mul(t1, y0, y0)
                nc.vector.tensor_mul(t1, t1, q)
                nc.vector.tensor_scalar(
                    t1, t1, -0.5, 1.5, AluOpType.mult, AluOpType.add
                )
                nc.vector.tensor_mul(rr, y0, t1)
                nc.vector.tensor_add(mu, m_e, m_o)
                nc.vector.tensor_scalar_mul(mu, mu, 0.5)

            # =================== PHASE 1 ===================
            # ACT funcs used: Exp, Relu, Tanh, Copy (all in exp_and_others).
            # Software-pipelined: prologue(ss+1) [loads/stats/applies/casts]
            # is emitted before body(ss) [transposes/matmuls/phi/gate] so each
            # engine's in-order stream interleaves the two supersteps.
            def p1_pro1(ss):
                """Loads + xp add + bf16 casts — no dependence on stats."""
                tsl = slice(ss * SS, (ss + 1) * SS)
                if ss == 0:
                    x_t, s_t, pos_t = pre_x, pre_s, pre_p
                else:
                    x_t = io.tile([P, NST, C], F32, name="xt", tag="xt")
                    nc.sync.dma_start(out=x_t, in_=x_d[tsl, :].rearrange("(s p) c -> p s c", p=P))
                    s_t = io.tile([P, NST, C], F32, name="st", tag="st", bufs=3)
                    nc.sync.dma_start(out=s_t, in_=s_d[tsl, :].rearrange("(s p) c -> p s c", p=P))
                    pos_t = io.tile([P, NST, C], F32, name="pt", tag="pt")
                    nc.sync.dma_start(out=pos_t, in_=pos_d[tsl, :].rearrange("(s p) c -> p s c", p=P))

                xp_v = xp_full[:, ss * NST:(ss + 1) * NST, :]
                nc.gpsimd.tensor_add(xp_v, x_t, pos_t)

                # raw x/s cast to bf16: transposes run 1 cyc/row, drains at
                # DVE 2x rate, and s-stats can use the 2x bn_stats path
                xb = wk.tile([P, NST, C], BF16, name="xb", tag="xb", bufs=3)
                nc.vector.tensor_copy(xb, x_t)
                sb_ = wk.tile([P, NST, C], BF16, name="sbb", tag="sbb", bufs=3)
                nc.vector.tensor_copy(sb_, s_t)
                return {"s_t": s_t, "xp_v": xp_v, "xb": xb, "sbb": sb_, "ss": ss}

            def p1_pro2(pro):
                ss = pro["ss"]
                s_t, xp_v, xb, sb_ = pro["s_t"], pro["xp_v"], pro["xb"], pro["sbb"]
                # LN1/LN2 stats (bf16 in/out enables the DVE 2x path; HW
                # BNStats requires exactly 6 output elements per call)
                stt = sm.tile([P, 2, NST, 6], BF16, name="stt", tag="stt")
                for st in range(NST):
                    nc.vector.bn_stats(stt[:, 0, st, :], xp_v[:, st, :])
                    nc.vector.bn_stats(stt[:, 1, st, :], sb_[:, st, :])
                mu = sm.tile([P, 2, NST], F32, name="mu", tag="mu")
                rr = sm.tile([P, 2, NST], F32, name="rr", tag="rr")
                stats_chain(
                    stt.rearrange("p a b c -> p (a b) c"),
                    mu.rearrange("p a b -> p (a b)"),
                    rr.rearrange("p a b -> p (a b)"),
                    2 * NST,
                )

                # LN applies -> bf16 token-major (on Pool: DVE is the bottleneck)
                x_ln = wk.tile([P, NST, C], BF16, name="xln", tag="xln", bufs=3)
                s_ln = wk.tile([P, NST, C], BF16, name="sln", tag="sln", bufs=3)
                for st in range(NST):
                    nc.gpsimd.tensor_scalar(
                        x_ln[:, st, :], xp_v[:, st, :],
                        mu[:, 0, st:st + 1], rr[:, 0, st:st + 1],
                        AluOpType.subtract, AluOpType.mult,
                    )
                    nc.gpsimd.tensor_scalar(
                        s_ln[:, st, :], s_t[:, st, :],
                        mu[:, 1, st:st + 1], rr[:, 1, st:st + 1],
                        AluOpType.subtract, AluOpType.mult,
                    )

                return {"x_ln": x_ln, "s_ln": s_ln, "xb": xb, "sbb": sb_}

            def p1_body(ss, pro):
                tsl = slice(ss * SS, (ss + 1) * SS)
                x_ln, s_ln = pro["x_ln"], pro["s_ln"]
                xb, sb_ = pro["xb"], pro["sbb"]
                # transposes: all 8 per-st blocks (x_ln/s_ln/x/s, bf16) into
                # ONE full-bank psum tile, drained with a single copy.
                # tboth layout [P, tensor(4), KC, SS]: 0=x_ln 1=s_ln 2=x 3=s
                tboth = wk.tile([P, 4, KC, SS], BF16, name="tboth", tag="tln", bufs=2)
                tln = tboth[:, 0:2]
                traw = tboth[:, 2:4]
                for st in range(NST):
                    tp8 = ptpa.tile([P, 8 * P], BF16, name="tp8", tag="tpA", bufs=2)
                    i = 0
                    for src in (x_ln, s_ln, xb, sb_):
                        for kc in range(KC):
                            nc.tensor.transpose(
                                tp8[:, i * P:(i + 1) * P],
                                src[:, st, kc * P:(kc + 1) * P],
                                ident,
                            )
                            i += 1
                    nc.vector.tensor_copy(
                        tboth[:, :, :, st * P:(st + 1) * P].rearrange("p a k t -> p (a k) t"),
                        tp8.rearrange("p (g t) -> p g t", g=8),
                    )

                # q channel-major + phi
                for oc in range(KC):
                    pq = pmm.tile([P, SS], F32, name="pq", tag="mm", bufs=4)
                    i = 0
                    for w_sb, ti in ((wqi, 0), (wqs, 1)):
                        for kc in range(KC):
                            nc.tensor.matmul(
                                pq,
                                lhsT=w_sb[:, kc, oc * P:(oc + 1) * P],
                                rhs=tln[:, ti, kc, :],
                                start=(i == 0), stop=(i == 3),
                                skip_group_check=True,
                            )
                            i += 1
                    tmin = wk.tile([P, SS], BF16, name="tmin", tag="tmin")
                    if bq_sb is not None:
                        nc.vector.tensor_scalar(
                            tmin, pq, bq_sb[:, oc:oc + 1], 0.0,
                            AluOpType.add, AluOpType.min,
                        )
                        trel = wk.tile([P, SS], BF16, name="trelq", tag="trelq")
                        nc.scalar.activation(trel, pq, AF.Relu, bias=bq_sb[:, oc:oc + 1])
                    else:
                        nc.vector.tensor_scalar_min(tmin, pq, 0.0)
                        trel = wk.tile([P, SS], BF16, name="trelq", tag="trelq")
                        nc.scalar.activation(trel, pq, AF.Relu)
                    texp = wk.tile([P, SS], BF16, name="texpq", tag="texpq")
                    nc.scalar.activation(texp, tmin, AF.Exp)
                    nc.vector.tensor_add(phiqT[:, oc, tsl], texp, trel)

                # k|v token-major + phi(k) + KV accumulation
                phik = wk.tile([P, NST, C], BF16, name="phik", tag="phik")
                v_sb = wk.tile([P, NST, C], BF16, name="vsb", tag="vsb")
                for st in range(NST):
                    pkvm = pmm.tile([P, 2 * C], F32, name="pkvm", tag="mm", bufs=4)
                    i = 0
                    for w_sb, ti in ((wqi, 0), (wqs, 1)):
                        for kc in range(KC):
                            nc.tensor.matmul(
                                pkvm,
                                lhsT=tln[:, ti, kc, st * P:(st + 1) * P],
                                rhs=w_sb[:, kc, C:3 * C],
                                start=(i == 0), stop=(i == 3),
                                skip_group_check=True,
                            )
                            i += 1
                    kmin = wk.tile([P, C], BF16, name="kmin", tag="kmin")
                    krel = wk.tile([P, C], BF16, name="krel", tag="krel")
                    if bkv_sb is not None:
                        kb = wk.tile([P, C], F32, name="kb", tag="kb")
                        nc.vector.tensor_add(kb, pkvm[:, 0:C], bkv_sb[:, 0:C])
                        nc.vector.tensor_scalar_min(kmin, kb, 0.0)
                        nc.scalar.activation(krel, kb, AF.Relu)
                        nc.vector.tensor_add(v_sb[:, st, :], pkvm[:, C:2 * C], bkv_sb[:, C:2 * C])
                    else:
                        nc.vector.tensor_scalar_min(kmin, pkvm[:, 0:C], 0.0)
                        nc.scalar.activation(krel, pkvm[:, 0:C], AF.Relu)
                        nc.scalar.copy(v_sb[:, st, :], pkvm[:, C:2 * C])
                    kexp = wk.tile([P, C], BF16, name="kexp", tag="kexp")
                    nc.scalar.activation(kexp, kmin, AF.Exp)
                    nc.vector.tensor_add(phik[:, st, :], kexp, krel)

                    for mc in range(KC):
                        nc.tensor.matmul(
                            kvps[mc],
                            lhsT=phik[:, st, mc * P:(mc + 1) * P],
                            rhs=v_sb[:, st, :],
                            start=False,
                            stop=(ss == NSS - 1 and st == NST - 1),
                            skip_group_check=True,
                        )
                if dbg and ss == NSS - 1:
                    nc.sync.dma_start(out=dbg_phik_d, in_=phik.rearrange("p a b -> p (a b)"))
                    nc.sync.dma_start(out=dbg_v_d, in_=v_sb.rearrange("p a b -> p (a b)"))

                # gate: u2 = tanh(0.5*(g+b)); upd = 0.5*u2 + 0.5 (batched)
                if os.environ.get("KSKIP_GATE", "0") == "1":
                    return
                updt = wk.tile([P, NST, C], BF16, name="updt", tag="updt")
                for st in range(NST):
                    pg = pmm.tile([P, SS], F32, name="pg", tag="mm", bufs=4)
                    i = 0
                    for ti, koff in ((0, 0), (1, 2)):
                        for kc in range(KC):
                            nc.tensor.matmul(
                                pg[:, 0:C],
                                lhsT=traw[:, ti, kc, st * P:(st + 1) * P],
                                rhs=wgate[:, koff + kc, :],
                                start=(i == 0), stop=(i == 3),
                                skip_group_check=True,
                            )
                            i += 1
                    if bgate_sb is not None:
                        gb = wk.tile([P, C], F32, name="gb", tag="gb")
                        nc.vector.tensor_add(gb, pg[:, 0:C], bgate_sb)
                        nc.scalar.activation(updt[:, st, :], gb, AF.Tanh)
                    else:
                        nc.scalar.activation(updt[:, st, :], pg[:, 0:C], AF.Tanh)
                nc.vector.tensor_scalar(
                    upd[:, ss * NST:(ss + 1) * NST, :], updt, 0.5, 0.5,
                    AluOpType.mult, AluOpType.add,
                )

            # 3-stage pipeline: pro1 two ahead, pro2 one ahead of body, so
            # Pool's in-order stream runs xp(ss+2) before applies(ss+1) and
            # stats/applies of ss+1 overlap body(ss).
            pr1 = [p1_pro1(0), p1_pro1(1)]
            cur = p1_pro2(pr1[0])
            for ss in range(NSS):
                if ss + 2 < NSS:
                    pr1.append(p1_pro1(ss + 2))
                nxt = p1_pro2(pr1[ss + 1]) if ss + 1 < NSS else None
                p1_body(ss, cur)
                cur = nxt

            # ---- KV block-diag mask, then fold proj: M = KV_bd @ wproj ----
            kvbd = pers.tile([P, KC, C], BF16, name="kvbd")
            for mc in range(KC):
                nc.vector.tensor_mul(kvbd[:, mc, :], kvps[mc], mask[:, mc, :])
            kvbdT = pers.tile([P, KC, C], BF16, name="kvbdt")
            for ec in range(KC):
                tpM = ptpa.tile([P, 4 * P], BF16, name="tpM", tag="tpA", bufs=2)
                for dc in range(KC):
                    nc.tensor.transpose(
                        tpM[:, dc * P:(dc + 1) * P],
                        kvbd[:, dc, ec * P:(ec + 1) * P],
                        ident,
                    )
                nc.vector.tensor_copy(kvbdT[:, ec, :], tpM[:, 0:2 * P])
            m_sb = pers.tile([P, KC, C], BF16, name="msb")
            for dc in range(KC):
                pm = pmm.tile([P, C], F32, name="pm", tag="mm", bufs=4)
                for ec in range(KC):
                    nc.tensor.matmul(
                        pm,
                        lhsT=kvbdT[:, ec, dc * P:(dc + 1) * P],
                        rhs=wproj[:, ec, :],
                        start=(ec == 0), stop=(ec == KC - 1),
                        skip_group_check=True,
                    )
                nc.vector.tensor_copy(m_sb[:, dc, :], pm)

            # =================== PHASE 2 ===================
            # ACT funcs used: Gelu, Copy (both in gelu_and_others).
            # Software-pipelined: stage_a(ss) [attn/o3/ns/LN3/transpose] is
            # emitted interleaved with stage_b(ss-1) [fc1/gelu/fc2/store] so
            # each engine's in-order stream alternates between supersteps.
            def stage_a(ss):
                tsl = slice(ss * SS, (ss + 1) * SS)
                s2 = io.tile([P, NST, C], F32, name="s2", tag="st", bufs=3)
                nc.sync.dma_start(out=s2, in_=s_d[tsl, :].rearrange("(s p) c -> p s c", p=P))

                xp_v = xp_full[:, ss * NST:(ss + 1) * NST, :]
                o3 = wk.tile([P, NST, C], F32, name="o3", tag="o3", bufs=2)
                ns_t = wk.tile([P, NST, C], F32, name="nst", tag="nst")
                for st in range(NST):
                    pa = pmm.tile([P, C], F32, name="pa", tag="mm", bufs=4)
                    for dc in range(KC):
                        nc.tensor.matmul(
                            pa,
                            lhsT=phiqT[:, dc, ss * SS + st * P:ss * SS + (st + 1) * P],
                            rhs=m_sb[:, dc, :],
                            start=(dc == 0), stop=(dc == KC - 1),
                            skip_group_check=True,
                        )
                    a_src = pa
                    if bproj_sb is not None:
                        ab = wk.tile([P, C], F32, name="ab", tag="ab")
                        nc.vector.tensor_add(ab, pa, bproj_sb)
                        a_src = ab
                    # out3 = xp + attn
                    nc.vector.scalar_tensor_tensor(
                        o3[:, st, :], a_src, 1.0, xp_v[:, st, :],
                        AluOpType.mult, AluOpType.add,
                    )
                    # ns = s + (attn - s)*upd
                    dt_ = wk.tile([P, C], F32, name="dt", tag="dt")
                    nc.vector.tensor_sub(dt_, a_src, s2[:, st, :])
                    du = wk.tile([P, C], F32, name="du", tag="du")
                    nc.gpsimd.tensor_mul(du, dt_, upd[:, ss * NST + st, :])
                    nc.gpsimd.tensor_add(ns_t[:, st, :], du, s2[:, st, :])
                nc.sync.dma_start(
                    out=ns_d[tsl, :].rearrange("(s p) c -> p s c", p=P), in_=ns_t
                )

                # LN3
                stt3 = sm.tile([P, NST, 6], F32, name="stt3", tag="stt")
                for st in range(NST):
                    nc.vector.bn_stats(stt3[:, st, :], o3[:, st, :])
                mu3 = sm.tile([P, NST], F32, name="mu3", tag="mu")
                rr3 = sm.tile([P, NST], F32, name="rr3", tag="rr")
                stats_chain(stt3, mu3, rr3, NST)
                h_ln = wk.tile([P, NST, C], BF16, name="hln", tag="xln", bufs=3)
                for st in range(NST):
                    nc.gpsimd.tensor_scalar(
                        h_ln[:, st, :], o3[:, st, :],
                        mu3[:, st:st + 1], rr3[:, st:st + 1],
                        AluOpType.subtract, AluOpType.mult,
                    )

                return {"o3": o3, "h_ln": h_ln}

            def stage_a2(carry):
                # transpose h_ln (kc-major placement within each psum tile);
                # emitted AFTER stage_b(ss-1) so PE doesn't stall on LN3(ss)
                h_ln = carry["h_ln"]
                hlnT = wk.tile([P, KC, SS], BF16, name="hlnt", tag="tln", bufs=2)
                for half in range(2):
                    tpC = ptpa.tile([P, 4 * P], BF16, name="tpC", tag="tpA", bufs=2)
                    for kc in range(KC):
                        for sti in range(2):
                            st = half * 2 + sti
                            nc.tensor.transpose(
                                tpC[:, (kc * 2 + sti) * P:(kc * 2 + sti + 1) * P],
                                h_ln[:, st, kc * P:(kc + 1) * P],
                                ident,
                            )
                    nc.scalar.copy(
                        hlnT[:, :, half * 2 * P:(half * 2 + 2) * P],
                        tpC.rearrange("p (k t) -> p k t", k=2),
                    )
                carry["hlnT"] = hlnT

            def stage_b(ss, carry):
                tsl = slice(ss * SS, (ss + 1) * SS)
                o3, hlnT = carry["o3"], carry["hlnT"]
                # fc1 + gelu (channel-major [HID, SS])
                h1gT = wk.tile([P, HC, SS], BF16, name="h1gt", tag="h1g")
                for ck in range(HC):
                    pf = pmm.tile([P, SS], F32, name="pf", tag="mm", bufs=4)
                    for kc in range(KC):
                        nc.tensor.matmul(
                            pf,
                            lhsT=wfc1[:, kc, ck * P:(ck + 1) * P],
                            rhs=hlnT[:, kc, :],
                            start=(kc == 0), stop=(kc == KC - 1),
                            skip_group_check=True,
                        )
                    if os.environ.get("KGELU_SIM", "0") == "1":
                        # CoreSim lacks Gelu: x*sigmoid(1.702x) approx
                        gt = wk.tile([P, SS], BF16, name="gt", tag="tmin")
                        nc.scalar.activation(gt, pf, AF.Sigmoid, scale=1.702)
                        nc.vector.tensor_mul(h1gT[:, ck, :], pf, gt)
                    elif bfc1_sb is not None:
                        nc.scalar.activation(h1gT[:, ck, :], pf, AF.Gelu, bias=bfc1_sb[:, ck:ck + 1])
                    else:
                        nc.scalar.activation(h1gT[:, ck, :], pf, AF.Gelu)

                # fc2 + final residual
                fin = wk.tile([P, NST, C], F32, name="fin", tag="fin")
                for st in range(NST):
                    po = pmm.tile([P, SS], F32, name="po", tag="mm", bufs=4)
                    for ck in range(HC):
                        nc.tensor.matmul(
                            po[:, 0:C],
                            lhsT=h1gT[:, ck, st * P:(st + 1) * P],
                            rhs=wfc2[:, ck, :],
                            start=(ck == 0), stop=(ck == HC - 1),
                            skip_group_check=True,
                        )
                    if bfc2_sb is not None:
                        fb = wk.tile([P, C], F32, name="fb", tag="dt")
                        nc.vector.tensor_add(fb, po[:, 0:C], bfc2_sb)
                        nc.vector.tensor_add(fin[:, st, :], fb, o3[:, st, :])
                    else:
                        nc.vector.scalar_tensor_tensor(
                            fin[:, st, :], po[:, 0:C], 1.0, o3[:, st, :],
                            AluOpType.mult, AluOpType.add,
                        )
                nc.sync.dma_start(
                    out=out_d[tsl, :].rearrange("(s p) c -> p s c", p=P), in_=fin
                )

            if os.environ.get("KSKIP_P2", "0") == "1":
                if dbg:
                    nc.sync.dma_start(out=dbg_phiqt_d, in_=phiqT.rearrange("p a b -> p (a b)"))
                    nc.sync.dma_start(out=dbg_kvbd_d, in_=kvbd.rearrange("p a b -> p (a b)"))
                    nc.sync.dma_start(out=dbg_m_d, in_=m_sb.rearrange("p a b -> p (a b)"))
                    nc.sync.dma_start(out=dbg_upd_d, in_=upd.rearrange("p a b -> p (a b)"))
                    nc.sync.dma_start(out=dbg_xp_d, in_=xp_full.rearrange("p a b -> p (a b)"))
                return nc
            carry = stage_a(0)
            stage_a2(carry)
            for ss in range(1, NSS):
                nxt = stage_a(ss)
                stage_b(ss - 1, carry)
                stage_a2(nxt)
                carry = nxt
            stage_b(NSS - 1, carry)

            if dbg:
                nc.sync.dma_start(out=dbg_phiqt_d, in_=phiqT.rearrange("p a b -> p (a b)"))
                nc.sync.dma_start(out=dbg_kvbd_d, in_=kvbd.rearrange("p a b -> p (a b)"))
                nc.sync.dma_start(out=dbg_m_d, in_=m_sb.rearrange("p a b -> p (a b)"))
                nc.sync.dma_start(out=dbg_upd_d, in_=upd.rearrange("p a b -> p (a b)"))
                nc.sync.dma_start(out=dbg_xp_d, in_=xp_full.rearrange("p a b -> p (a b)"))

    return nc


def kernel(**inputs):
    global LAST_RESULT
    f = lambda k: np.ascontiguousarray(np.asarray(inputs[k], dtype=np.float32))
    input_ = f("input_")
    prev_state = f("prev_state")
    pos_embed = f("pos_embed")
    n1w, n1b = f("norm1_w"), f("norm1_b")
    n2w, n2b = f("norm2_w"), f("norm2_b")
    n3w, n3b = f("norm3_w"), f("norm3_b")
    qkv_i, qkv_s = f("qkv_input_w"), f("qkv_state_w")
    proj_w, proj_b = f("proj_w"), f("proj_b")
    gate_w, gate_b = f("gate_w"), f("gate_b")
    fc1_w, fc1_b = f("fc1_w"), f("fc1_b")
    fc2_w, fc2_b = f("fc2_w"), f("fc2_b")

    bf = mybir.dt.np(BF16)
    sqC = float(np.sqrt(C))  # 16; LN applies produce x_ln/sqrt(C)

    # Fold LN affine + sqrt(C) into consuming weights (host-side):
    w_qi = np.ascontiguousarray((qkv_i * n1w[None, :]).T * sqC).astype(bf)
    w_qs = np.ascontiguousarray((qkv_s * n2w[None, :]).T * sqC).astype(bf)
    b_qkv = n1b @ qkv_i.T + n2b @ qkv_s.T
    w_fc1 = np.ascontiguousarray((fc1_w * n3w[None, :]).T * sqC).astype(bf)
    b_fc1 = fc1_b + n3b @ fc1_w.T
    w_proj = np.ascontiguousarray(proj_w.T).astype(bf)
    w_gate = np.ascontiguousarray(gate_w.T * 0.5).astype(bf)  # tanh half-angle
    w_fc2 = np.ascontiguousarray(fc2_w.T).astype(bf)

    mask = np.zeros((C, C), dtype=np.float32)
    for h in range(H):
        mask[h * D:(h + 1) * D, h * D:(h + 1) * D] = 1.0

    nz = {
        "b_q": bool(np.any(b_qkv[:C])),
        "b_kv": bool(np.any(b_qkv[C:])),
        "b_proj": bool(np.any(proj_b)),
        "b_gate": bool(np.any(gate_b)),
        "b_fc1": bool(np.any(b_fc1)),
        "b_fc2": bool(np.any(fc2_b)),
    }

    nc = _build(nz)

    pos = np.ascontiguousarray(pos_embed[0])
    base = {
        "pos": pos, "w_qi": w_qi, "w_qs": w_qs, "w_proj": w_proj,
        "w_gate": w_gate, "w_fc1": w_fc1, "w_fc2": w_fc2,
        "mask": mask.astype(bf),
        "ident": np.eye(P, dtype=np.float32).astype(bf),
        "magic": np.full((16,), MAGIC, dtype=np.int32),
    }
    if nz["b_q"]:
        base["b_q"] = np.ascontiguousarray(b_qkv[:C])
    if nz["b_kv"]:
        base["b_kv"] = np.ascontiguousarray(b_qkv[C:])
    if nz["b_proj"]:
        base["b_proj"] = proj_b
    if nz["b_gate"]:
        base["b_gate"] = gate_b * 0.5
    if nz["b_fc1"]:
        base["b_fc1"] = np.ascontiguousarray(b_fc1)
    if nz["b_fc2"]:
        base["b_fc2"] = fc2_b

    in_maps = [
        {**base, "x": input_[b], "s": prev_state[b]} for b in range(B)
    ]

    if not nc.is_finalized():
        nc.finalize()

    res = run_bass_kernel_spmd(nc, in_maps, list(range(B)))
    LAST_RESULT = res
    output = np.stack([res.results[b]["out"] for b in range(B)])
    new_state = np.stack([res.results[b]["ns"] for b in range(B)])
    return output, new_state
